# revision 4
# baseline (speedup 1.0000x reference)
"""Trainium2 Bass kernel for nn_ExportableGENConv (GENConv message passing +
channelwise softmax aggregation + MLP with global-batch BatchNorm), sharded
across 8 NeuronCores.

Contract: kernel(**inputs) takes the FULL inputs of reference.setup_inputs()
and returns the FULL [32768, 64] float32 output.

Sharding: nodes (each with K=32 contiguous incoming edge slots) are split
across 8 cores. Per-edge source features x[src] are materialized host-side
during staging (the halo exchange) in fp16; the per-edge message + per-node
softmax + MLP run on device. Global BatchNorm statistics are combined on host
between two NEFF launches (in-kernel collectives fail to load under this
runtime): phase 1 produces h1 = (aggregated+x) @ W1 (fp16) plus per-core
sum/sumsq, phase 2 applies the batch-norm affine + ReLU + W2.

Math (per node i, channel h, over valid slots k):
  t = x[src] + ea @ W_edge            (invalid slots: staged x = -3e4 -> t << 0)
  reference: m = relu(t) + 1e-7; softmax over k of m; res = sum m*alpha.
  With r = relu(t):  res = (sum_k r*e^r) / (sum_k e^r + 1e-16) + 1e-7
  Device: E = exp(t); P = max(E, 1) (= e^r; invalid slots contribute exactly
  1.0, removed via a host-staged per-node count); Pm = max(t, 0)*E (= r*e^r;
  invalid slots contribute exactly 0).

Device layout: tile = 64 nodes = 2048 edge slots as [128, 1024] with
partition p = (node%32)*4 + (k%4) and free = (c2 = (node%64)//32, b = k//4,
h).  The per-node k-reduction: j = k%4 via a block-diagonal ones stationary
(PE), b via 8 accumulating 64-row matmuls into a per-octet PSUM accumulator
(no DVE reduce).  All staged data fp16; PSUM accumulation fp32.
"""

import numpy as np
from contextlib import ExitStack

import concourse.bass as bass
import concourse.tile as tile
from concourse import mybir
from concourse.bass_utils import run_bass_kernel_spmd

# ---------------------------------------------------------------- constants
N, K, H, ED = 32768, 32, 64, 32
E = N * K
NCORES = 8
NPC = N // NCORES            # nodes per core = 4096
NTT = NPC // 64              # 64-node tiles per core = 64
TCH = 4                      # tiles per DMA chunk (256 nodes, 1.5 MB)
NCH = NTT // TCH             # chunks = 16
NOCT = 4                     # octets (1024 nodes each)
NEG_BIG = -30000.0           # fp16-safe "minus infinity" for invalid slots

_compiled = {}


# ------------------------------------------------------- multi-wait legalizer
def _legalize_multiwaits(nc):
    """This walrus build accepts only ONE sync wait per instruction; move the
    excess onto injected same-engine drain carriers placed immediately before
    the instruction (semantics-preserving: the engine stalls there instead)."""
    n_injected = 0
    for fn in nc.m.functions:
        for blk in fn.blocks:
            bb = blk if hasattr(blk, "instructions") else blk.bb
            insts = list(bb.instructions)
            out = []
            for inst in insts:
                si = inst.sync_info
                if si is not None and si.on_wait and len(si.on_wait) > 1:
                    waits = list(si.on_wait)
                    for w in waits[:-1]:
                        nop = mybir.InstDrain(
                            name=f"waitfix-{nc.next_id()}", ins=[], outs=[]
                        )
                        nop.engine = inst.engine
                        nop.sync_info = mybir.SyncInfo(on_wait=[w], on_update=[])
                        nc.register_instruction(nop, overwrite=True)
                        out.append(nop)
                        n_injected += 1
                    inst.sync_info = mybir.SyncInfo(
                        on_wait=[waits[-1]], on_update=list(si.on_update or [])
                    )
                out.append(inst)
            if len(out) != len(insts):
                bb.instructions = out
    return n_injected


# ------------------------------------------------------------ phase-1 kernel
def _build_phase1():
    fp32 = mybir.dt.float32
    fp16 = mybir.dt.float16
    Act = mybir.ActivationFunctionType
    Alu = mybir.AluOpType
    nc = bass.Bass()

    ed_d = nc.declare_dram_parameter("edges", [128, NTT * 1536], fp16, isOutput=False)
    wbd_d = nc.declare_dram_parameter("wbd", [128, 256], fp16, isOutput=False)
    bd_d = nc.declare_dram_parameter("bd", [128, 32], fp16, isOutput=False)
    id_d = nc.declare_dram_parameter("ident", [128, 128], fp16, isOutput=False)
    w1_d = nc.declare_dram_parameter("w1", [64, 128], fp16, isOutput=False)
    corr_d = nc.declare_dram_parameter("corr", [128, 2048], fp32, isOutput=False)
    xres_d = nc.declare_dram_parameter("xres", [128, 2048], fp16, isOutput=False)
    h1_d = nc.declare_dram_parameter("h1", [128, NPC], fp16, isOutput=True)
    st_d = nc.declare_dram_parameter("stats", [128, 2], fp32, isOutput=True)

    with tile.TileContext(nc) as tc, ExitStack() as ctx:
        const = ctx.enter_context(tc.tile_pool(name="const", bufs=1))
        sb = ctx.enter_context(tc.tile_pool(name="sb", bufs=1))
        ed_p = ctx.enter_context(tc.tile_pool(name="ed", bufs=3))
        t16_p = ctx.enter_context(tc.tile_pool(name="t16", bufs=3))
        e_p = ctx.enter_context(tc.tile_pool(name="ee", bufs=3))
        pp_p = ctx.enter_context(tc.tile_pool(name="pp", bufs=3))
        pm_p = ctx.enter_context(tc.tile_pool(name="pm", bufs=3))
        cmb_p = ctx.enter_context(tc.tile_pool(name="cmb", bufs=2))
        outt_p = ctx.enter_context(tc.tile_pool(name="outt", bufs=2))
        h1s_p = ctx.enter_context(tc.tile_pool(name="h1s", bufs=2))
        sq_p = ctx.enter_context(tc.tile_pool(name="sq", bufs=2))
        stp_p = ctx.enter_context(tc.tile_pool(name="stp", bufs=1))
        ps_t = ctx.enter_context(tc.tile_pool(name="ps_t", bufs=2, space="PSUM"))
        ps_r = ctx.enter_context(tc.tile_pool(name="ps_r", bufs=2, space="PSUM"))
        ps_x = ctx.enter_context(tc.tile_pool(name="ps_x", bufs=2, space="PSUM"))

        wbd_t = const.tile([128, 256], fp16)
        nc.sync.dma_start(wbd_t[:], wbd_d[:])
        bd_t = const.tile([128, 32], fp16)
        nc.sync.dma_start(bd_t[:], bd_d[:])
        id_t = const.tile([128, 128], fp16)
        nc.sync.dma_start(id_t[:], id_d[:])
        w1_t = const.tile([64, 128], fp16)
        nc.sync.dma_start(w1_t[:], w1_d[:])
        corr_t = const.tile([128, 2048], fp32)
        nc.sync.dma_start(corr_t[:], corr_d[:])
        xres_t = const.tile([128, 2048], fp16)
        nc.sync.dma_start(xres_t[:], xres_d[:])

        # prologue: make PE observe const DMA sems via tiny touch matmuls
        pro_ps = ps_x.tile([128, 512], fp32, tag="ps_x")
        nc.tensor.matmul(pro_ps[0:8, 0:8], id_t[:, 0:8], id_t[:, 0:8],
                         start=True, stop=True, skip_group_check=True)
        nc.tensor.matmul(pro_ps[0:8, 0:8], wbd_t[:, 0:8], wbd_t[:, 0:8],
                         start=True, stop=True, skip_group_check=True)
        nc.tensor.matmul(pro_ps[0:8, 0:8], bd_t[:, 0:8], bd_t[:, 0:8],
                         start=True, stop=True, skip_group_check=True)
        nc.tensor.matmul(pro_ps[0:8, 0:8], w1_t[:, 0:8], w1_t[:, 0:8],
                         start=True, stop=True, skip_group_check=True)

        # stats partials (filled per octet)
        s1_parts = [stp_p.tile([128, 1], fp32, name=f"s1p_{i}") for i in range(2 * NOCT)]
        s2_parts = [stp_p.tile([128, 1], fp32, name=f"s2p_{i}") for i in range(2 * NOCT)]

        # ---- edge phase
        ed_t = None
        st_pair = [None, None]
        for T in range(NTT):
            ch, lane = divmod(T, TCH)
            oct_, t_in_oct = divmod(T, 16)
            if lane == 0:
                ed_t = ed_p.tile([128, TCH * 1536], fp16, tag="ed")
                nc.sync.dma_start(
                    ed_t[:], ed_d[:, ch * TCH * 1536:(ch + 1) * TCH * 1536])
            base = lane * 1536
            xs_v = ed_t[:, base:base + 1024]
            ea_v = ed_t[:, base + 1024:base + 1536].rearrange(
                "p (q e) -> p q e", q=4)

            pe_add = (T % 8) in (0, 3, 6)  # tiles adding x[src] on PE (else DVE)
            t_ps = ps_t.tile([128, 1024], fp32, tag="ps_t")
            for c2 in range(2):
                if pe_add:
                    nc.tensor.matmul(t_ps[:, 512 * c2:512 * c2 + 512], id_t[:],
                                     xs_v[:, 512 * c2:512 * c2 + 512],
                                     start=True, stop=False,
                                     skip_group_check=True)
                for g in range(2):
                    q = 2 * c2 + g
                    nc.tensor.matmul(
                        t_ps[:, 512 * c2 + 256 * g:512 * c2 + 256 * (g + 1)],
                        ea_v[:, q, :], wbd_t[:],
                        start=(not pe_add) and g == 0, stop=(g == 1),
                        skip_group_check=True)

            E_t = e_p.tile([128, 1024], fp16, tag="ee")
            Pm_t = pm_p.tile([128, 1024], fp16, tag="pm")
            if pe_add:
                # t lives (complete) in PSUM
                nc.scalar.activation(E_t[:], t_ps[:], Act.Exp)
                nc.vector.scalar_tensor_tensor(
                    Pm_t[:], t_ps[:], 0.0, E_t[:], op0=Alu.max, op1=Alu.mult)
            else:
                t16 = t16_p.tile([128, 1024], fp16, tag="t16")
                nc.vector.tensor_tensor(out=t16[:], in0=t_ps[:], in1=xs_v,
                                        op=Alu.add)
                nc.scalar.activation(E_t[:], t16[:], Act.Exp)
                nc.vector.scalar_tensor_tensor(
                    Pm_t[:], t16[:], 0.0, E_t[:], op0=Alu.max, op1=Alu.mult)
            P_t = pp_p.tile([128, 1024], fp16, tag="pp")
            nc.gpsimd.tensor_scalar_max(P_t[:], E_t[:], 1.0)

            # ---- per-node reduction into per-octet PSUM accumulators
            if t_in_oct == 0:
                st_pair = [ps_r.tile([128, 512], fp32, tag="ps_r",
                                     name=f"S_{oct_}"),
                           ps_r.tile([128, 512], fp32, tag="ps_r",
                                     name=f"T_{oct_}")]
            S_oct, T_oct = st_pair
            g8 = t_in_oct // 2
            Pv = P_t[:].rearrange("p (c b h) -> p c b h", c=2, b=8)
            Pmv = Pm_t[:].rearrange("p (c b h) -> p c b h", c=2, b=8)
            for c2 in range(2):
                c = 2 * (T % 2) + c2
                for b in range(8):
                    nc.tensor.matmul(
                        S_oct[32 * c:32 * c + 32, 64 * g8:64 * (g8 + 1)],
                        bd_t[:], Pv[:, c2, b, :],
                        start=(b == 0), stop=(b == 7),
                        tile_position=(0, 32 * c), skip_group_check=True)
                for b in range(8):
                    nc.tensor.matmul(
                        T_oct[32 * c:32 * c + 32, 64 * g8:64 * (g8 + 1)],
                        bd_t[:], Pmv[:, c2, b, :],
                        start=(b == 0), stop=(b == 7),
                        tile_position=(0, 32 * c), skip_group_check=True)

            # ---- octet tail: combine + transpose + h1 + stats
            if t_in_oct == 15:
                o = oct_
                den = cmb_p.tile([128, 512], fp32, tag="cmb", name=f"den{o}")
                nc.vector.tensor_tensor(
                    out=den[:], in0=S_oct[:],
                    in1=corr_t[:, 512 * o:512 * (o + 1)], op=Alu.subtract)
                rec = cmb_p.tile([128, 512], fp32, tag="cmb", name=f"rec{o}")
                nc.vector.reciprocal(rec[:], den[:])
                wres = cmb_p.tile([128, 512], fp16, tag="cmb", name=f"wr{o}")
                nc.vector.tensor_tensor(out=wres[:], in0=T_oct[:], in1=rec[:],
                                        op=Alu.mult)
                res = cmb_p.tile([128, 512], fp16, tag="cmb", name=f"res{o}")
                nc.gpsimd.tensor_tensor(
                    out=res[:], in0=wres[:],
                    in1=xres_t[:, 512 * o:512 * (o + 1)], op=Alu.add)

                outT = outt_p.tile([64, 1024], fp16, tag="outt")
                for j2 in range(2):
                    tr_ps = ps_x.tile([64, 512], fp16, tag="ps_x")
                    for gg in range(4):
                        g8b = 4 * j2 + gg
                        nc.tensor.transpose(
                            tr_ps[0:64, 128 * gg:128 * (gg + 1)],
                            res[:, 64 * g8b:64 * (g8b + 1)], id_t[:])
                    nc.scalar.copy(outT[:, 512 * j2:512 * (j2 + 1)],
                                   tr_ps[0:64, :])

                h1sb = h1s_p.tile([128, 1024], fp16, tag="h1s")
                for j2 in range(2):
                    h1_ps = ps_x.tile([128, 512], fp32, tag="ps_x")
                    nc.tensor.matmul(h1_ps[:], w1_t[:],
                                     outT[:, 512 * j2:512 * (j2 + 1)],
                                     start=True, stop=True,
                                     skip_group_check=True)
                    nc.scalar.activation(
                        h1sb[:, 512 * j2:512 * (j2 + 1)], h1_ps[:], Act.Copy,
                        accum_out=s1_parts[2 * o + j2][:])
                    sq = sq_p.tile([128, 512], fp16, tag="sq")
                    nc.scalar.activation(
                        sq[:], h1_ps[:], Act.Square,
                        accum_out=s2_parts[2 * o + j2][:])
                nc.scalar.dma_start(h1_d[:, 1024 * o:1024 * (o + 1)], h1sb[:])

        # ---- stats: sum the per-octet partials, write [128, 2]
        stats = sb.tile([128, 2], fp32)
        acc1 = sb.tile([128, 1], fp32)
        acc2 = sb.tile([128, 1], fp32)
        nc.vector.tensor_tensor(out=acc1[:], in0=s1_parts[0][:],
                                in1=s1_parts[1][:], op=Alu.add)
        nc.vector.tensor_tensor(out=acc2[:], in0=s2_parts[0][:],
                                in1=s2_parts[1][:], op=Alu.add)
        for i in range(2, 2 * NOCT):
            nc.vector.tensor_tensor(out=acc1[:], in0=acc1[:],
                                    in1=s1_parts[i][:], op=Alu.add)
            nc.vector.tensor_tensor(out=acc2[:], in0=acc2[:],
                                    in1=s2_parts[i][:], op=Alu.add)
        nc.vector.tensor_copy(stats[:, 0:1], acc1[:])
        nc.vector.tensor_copy(stats[:, 1:2], acc2[:])
        nc.scalar.dma_start(st_d[:], stats[:])

    _legalize_multiwaits(nc)
    return nc


# ------------------------------------------------------------ phase-2 kernel
def _build_phase2():
    fp32 = mybir.dt.float32
    fp16 = mybir.dt.float16
    Act = mybir.ActivationFunctionType
    nc = bass.Bass()

    h1_d = nc.declare_dram_parameter("h1", [128, NPC], fp16, isOutput=False)
    ss_d = nc.declare_dram_parameter("ss", [128, 2], fp32, isOutput=False)
    w2_d = nc.declare_dram_parameter("w2", [128, 64], fp16, isOutput=False)
    out_d = nc.declare_dram_parameter("out", [64, NPC], fp16, isOutput=True)

    with tile.TileContext(nc) as tc, ExitStack() as ctx:
        const = ctx.enter_context(tc.tile_pool(name="const", bufs=1))
        sb = ctx.enter_context(tc.tile_pool(name="sb", bufs=1))
        ps = ctx.enter_context(tc.tile_pool(name="ps", bufs=4, space="PSUM"))

        ss_t = const.tile([128, 2], fp32)
        nc.sync.dma_start(ss_t[:], ss_d[:])
        w2_t = const.tile([128, 64], fp16)
        nc.sync.dma_start(w2_t[:], w2_d[:])
        h1 = sb.tile([128, NPC], fp16)
        nc.sync.dma_start(h1[:], h1_d[:])

        h2 = sb.tile([128, NPC], fp16)
        nc.scalar.activation(h2[:], h1[:], Act.Relu, bias=ss_t[:, 1:2],
                             scale=ss_t[:, 0:1])
        out_sb = sb.tile([64, NPC], fp16)
        for j in range(NPC // 512):
            o_ps = ps.tile([64, 512], fp32, tag="ps")
            nc.tensor.matmul(o_ps[:], w2_t[:], h2[:, j * 512:(j + 1) * 512],
                             start=True, stop=True, skip_group_check=True)
            if j % 2 == 0:
                nc.scalar.copy(out_sb[:, j * 512:(j + 1) * 512], o_ps[:])
            else:
                nc.vector.tensor_copy(out_sb[:, j * 512:(j + 1) * 512], o_ps[:])
        nc.sync.dma_start(out_d[:], out_sb[:])

    _legalize_multiwaits(nc)
    return nc


# -------------------------------------------------------------- host staging
def _stage_core(x_c, xs_slot_c, ea_slot_c, ninv_c):
    """xs_slot_c: [npc, K, H] f32 (x[src], invalid slots = NEG_BIG)
    ea_slot_c: [npc, K, ED] f32;  ninv_c: [npc] f32.
    Returns edges [128, NTT*1536] f16, corr [128,2048] f32, xres [128,2048] f16.

    Device tile = 64 nodes: partition p = 4*(node%32) + k%4, free:
      xs block  [T, cols 0:1024)   = (c2=(node%64)//32, b=k//4, h)
      ea block  [T, cols 1024:1536) = (c2, g=b//4, (node%32, k%4)) on
                partitions (r=b%4, d)
    """
    a = xs_slot_c.reshape(NTT, 2, 32, 8, 4, H)          # [T, c2, m2, b, j, h]
    xs_part = np.ascontiguousarray(
        a.transpose(2, 4, 0, 1, 3, 5)).reshape(128, NTT, 1024)

    e = ea_slot_c.reshape(NTT, 2, 32, 2, 4, 4, ED)      # [T, c2, m2, g, r, j, d]
    ea_part = np.ascontiguousarray(
        e.transpose(4, 6, 0, 1, 3, 2, 5)).reshape(128, NTT, 512)

    edges = np.concatenate(
        [xs_part, ea_part], axis=2).reshape(128, NTT * 1536).astype(np.float16)

    # node n = 128*G + p'  (p' = n % 128)
    corr = (ninv_c.astype(np.float32) - 1e-16)[:, None] * np.ones((1, H), np.float32)
    corr_dev = np.ascontiguousarray(
        corr.reshape(NPC // 128, 128, H).transpose(1, 0, 2)).reshape(128, -1)
    xres_dev = np.ascontiguousarray(
        (x_c + 1e-7).reshape(NPC // 128, 128, H).transpose(1, 0, 2)
    ).reshape(128, -1).astype(np.float16)
    return edges, corr_dev, xres_dev


def _consts(W_edge):
    Wbd = np.zeros((128, 256), np.float16)
    for r in range(4):
        Wbd[32 * r:32 * r + 32, 64 * r:64 * r + 64] = W_edge.astype(np.float16)
    BD = np.zeros((128, 32), np.float16)
    for m in range(32):
        BD[4 * m:4 * m + 4, m] = 1.0
    ident = np.eye(128, dtype=np.float16)
    return Wbd, BD, ident


def kernel(x, edge_index, edge_attr, nbr, W_edge, W1, gamma, beta, W2):
    x = np.ascontiguousarray(np.asarray(x, dtype=np.float32))
    edge_index = np.asarray(edge_index)
    edge_attr = np.ascontiguousarray(np.asarray(edge_attr, dtype=np.float32))
    nbr = np.asarray(nbr)
    W_edge = np.ascontiguousarray(np.asarray(W_edge, dtype=np.float32))
    W1 = np.ascontiguousarray(np.asarray(W1, dtype=np.float32))
    gamma = np.asarray(gamma, dtype=np.float32)
    beta = np.asarray(beta, dtype=np.float32)
    W2 = np.ascontiguousarray(np.asarray(W2, dtype=np.float32))

    src = np.asarray(edge_index[0], dtype=np.int64)
    valid = nbr >= 0                                    # [N, K]
    expect = np.arange(E, dtype=np.int64).reshape(N, K)
    assert np.array_equal(np.where(valid, nbr, expect), expect), \
        "kernel assumes nbr[i,k] == i*K+k on valid slots"

    src_slot = src.reshape(N, K)
    xs_slot = x[src_slot]                               # host halo: [N, K, H]
    xs_slot[~valid] = NEG_BIG
    ninv = (~valid).sum(axis=1).astype(np.float32)      # [N]
    ea_slot = edge_attr.reshape(N, K, ED)

    Wbd, BD, ident = _consts(W_edge)

    if "p1" not in _compiled:
        _compiled["p1"] = _build_phase1()
        _compiled["p2"] = _build_phase2()

    in_maps = []
    for core in range(NCORES):
        sl = slice(core * NPC, (core + 1) * NPC)
        edges, corr_dev, xres_dev = _stage_core(
            x[sl], xs_slot[sl], ea_slot[sl], ninv[sl])
        in_maps.append({
            "edges": edges, "wbd": Wbd, "bd": BD, "ident": ident,
            "w1": W1.astype(np.float16), "corr": corr_dev, "xres": xres_dev,
        })

    res1 = run_bass_kernel_spmd(_compiled["p1"], in_maps,
                                core_ids=list(range(NCORES)))

    # host: combine BN stats (tiny 128-vector arithmetic), build scale/shift
    s1 = np.zeros(2 * H, np.float64)
    s2 = np.zeros(2 * H, np.float64)
    for core in range(NCORES):
        st = res1.results[core]["stats"].astype(np.float64)
        s1 += st[:, 0]
        s2 += st[:, 1]
    mean = (s1 / N).astype(np.float32)
    var = (s2 / N).astype(np.float32) - mean * mean
    scale = gamma / np.sqrt(var + 1e-5)
    shift = beta - mean * scale
    ss = np.stack([scale, shift], axis=1).astype(np.float32)  # [128, 2]

    in_maps2 = [{"h1": res1.results[core]["h1"], "ss": ss,
                 "w2": W2.astype(np.float16)}
                for core in range(NCORES)]
    res2 = run_bass_kernel_spmd(_compiled["p2"], in_maps2,
                                core_ids=list(range(NCORES)))

    out = np.empty((N, H), np.float32)
    for core in range(NCORES):
        out[core * NPC:(core + 1) * NPC] = \
            res2.results[core]["out"].astype(np.float32).T
    return out


# revision 6
# speedup vs baseline: 5.2595x; 5.2595x over previous
"""Trainium2 Bass kernel for nn_ExportableGENConv (GENConv message passing +
channelwise softmax aggregation + MLP with global-batch BatchNorm), sharded
across 8 NeuronCores.

Contract: kernel(**inputs) takes the FULL inputs of reference.setup_inputs()
and returns the FULL [32768, 64] float32 output.

Sharding: nodes (each with K=32 contiguous incoming edge slots) are split
across 8 cores. Per-edge source features x[src] are materialized host-side
during staging (the halo exchange) in fp16; the per-edge message + per-node
softmax + MLP run on device. Global BatchNorm statistics are combined on host
between two NEFF launches (in-kernel collectives fail to load under this
runtime): phase 1 produces h1 = (aggregated+x) @ W1 (fp16) plus per-core
sum/sumsq, phase 2 applies the batch-norm affine + ReLU + W2.

Math (per node i, channel h, over valid slots k):
  t = x[src] + ea @ W_edge            (invalid slots: staged x = -3e4 -> t << 0)
  reference: m = relu(t) + 1e-7; softmax over k of m; res = sum m*alpha.
  With r = relu(t):  res = (sum_k r*e^r) / (sum_k e^r + 1e-16) + 1e-7
  Device: E = exp(t); P = max(E, 1) (= e^r; invalid slots contribute exactly
  1.0, removed via a host-staged per-node count); Pm = max(t, 0)*E (= r*e^r;
  invalid slots contribute exactly 0).

Device layout: tile = 64 nodes = 2048 edge slots as [128, 1024] with
partition p = (node%32)*4 + (k%4) and free = (c2 = (node%64)//32, b = k//4,
h).  The per-node k-reduction: j = k%4 via a block-diagonal ones stationary
(PE), b via 8 accumulating 64-row matmuls into a per-octet PSUM accumulator
(no DVE reduce).  All staged data fp16; PSUM accumulation fp32.
"""

import numpy as np
from contextlib import ExitStack

import concourse.bass as bass
import concourse.tile as tile
from concourse import mybir
from concourse.bass_utils import run_bass_kernel_spmd

# ---------------------------------------------------------------- constants
N, K, H, ED = 32768, 32, 64, 32
E = N * K
NCORES = 8
NPC = N // NCORES            # nodes per core = 4096
NTT = NPC // 64              # 64-node tiles per core = 64
TCH = 4                      # tiles per DMA chunk (256 nodes, 1.5 MB)
NCH = NTT // TCH             # chunks = 16
NOCT = 4                     # octets (1024 nodes each)
NEG_BIG = -30000.0           # fp16-safe "minus infinity" for invalid slots

_compiled = {}


# ------------------------------------------------------- multi-wait legalizer
def _legalize_multiwaits(nc):
    """This walrus build accepts only ONE sync wait per instruction; move the
    excess onto injected same-engine drain carriers placed immediately before
    the instruction (semantics-preserving: the engine stalls there instead)."""
    n_injected = 0
    for fn in nc.m.functions:
        for blk in fn.blocks:
            bb = blk if hasattr(blk, "instructions") else blk.bb
            insts = list(bb.instructions)
            out = []
            for inst in insts:
                si = inst.sync_info
                if si is not None and si.on_wait and len(si.on_wait) > 1:
                    waits = list(si.on_wait)
                    for w in waits[:-1]:
                        nop = mybir.InstDrain(
                            name=f"waitfix-{nc.next_id()}", ins=[], outs=[]
                        )
                        nop.engine = inst.engine
                        nop.sync_info = mybir.SyncInfo(on_wait=[w], on_update=[])
                        nc.register_instruction(nop, overwrite=True)
                        out.append(nop)
                        n_injected += 1
                    inst.sync_info = mybir.SyncInfo(
                        on_wait=[waits[-1]], on_update=list(si.on_update or [])
                    )
                out.append(inst)
            if len(out) != len(insts):
                bb.instructions = out
    return n_injected


# ------------------------------------------------------------ phase-1 kernel
def _build_phase1():
    fp32 = mybir.dt.float32
    fp16 = mybir.dt.float16
    Act = mybir.ActivationFunctionType
    Alu = mybir.AluOpType
    nc = bass.Bass()

    ed_d = nc.declare_dram_parameter("edges", [128, NTT * 1536], fp16, isOutput=False)
    wbd_d = nc.declare_dram_parameter("wbd", [128, 256], fp16, isOutput=False)
    bd_d = nc.declare_dram_parameter("bd", [128, 32], fp16, isOutput=False)
    id_d = nc.declare_dram_parameter("ident", [128, 128], fp16, isOutput=False)
    w1_d = nc.declare_dram_parameter("w1", [64, 128], fp16, isOutput=False)
    corr_d = nc.declare_dram_parameter("corr", [128, 2048], fp32, isOutput=False)
    xres_d = nc.declare_dram_parameter("xres", [128, 2048], fp16, isOutput=False)
    h1_d = nc.declare_dram_parameter("h1", [128, NPC], fp16, isOutput=True)
    st_d = nc.declare_dram_parameter("stats", [128, 2], fp32, isOutput=True)

    with tile.TileContext(nc) as tc, ExitStack() as ctx:
        const = ctx.enter_context(tc.tile_pool(name="const", bufs=1))
        sb = ctx.enter_context(tc.tile_pool(name="sb", bufs=1))
        ed_p = ctx.enter_context(tc.tile_pool(name="ed", bufs=3))
        r_p = ctx.enter_context(tc.tile_pool(name="rr", bufs=3))
        pp_p = ctx.enter_context(tc.tile_pool(name="pp", bufs=3))
        pm_p = ctx.enter_context(tc.tile_pool(name="pm", bufs=3))
        cmb_p = ctx.enter_context(tc.tile_pool(name="cmb", bufs=2))
        outt_p = ctx.enter_context(tc.tile_pool(name="outt", bufs=2))
        h1s_p = ctx.enter_context(tc.tile_pool(name="h1s", bufs=2))
        sq_p = ctx.enter_context(tc.tile_pool(name="sq", bufs=2))
        stp_p = ctx.enter_context(tc.tile_pool(name="stp", bufs=1))
        ps_t = ctx.enter_context(tc.tile_pool(name="ps_t", bufs=2, space="PSUM"))
        ps_s = ctx.enter_context(tc.tile_pool(name="ps_s", bufs=1, space="PSUM"))
        ps_u = ctx.enter_context(tc.tile_pool(name="ps_u", bufs=1, space="PSUM"))
        ps_x = ctx.enter_context(tc.tile_pool(name="ps_x", bufs=2, space="PSUM"))

        wbd_t = const.tile([128, 256], fp16)
        nc.sync.dma_start(wbd_t[:], wbd_d[:])
        bd_t = const.tile([128, 32], fp16)
        nc.sync.dma_start(bd_t[:], bd_d[:])
        id_t = const.tile([128, 128], fp16)
        nc.sync.dma_start(id_t[:], id_d[:])
        w1_t = const.tile([64, 128], fp16)
        nc.sync.dma_start(w1_t[:], w1_d[:])
        corr_t = const.tile([128, 2048], fp32)
        nc.sync.dma_start(corr_t[:], corr_d[:])
        xres_t = const.tile([128, 2048], fp16)
        nc.sync.dma_start(xres_t[:], xres_d[:])

        # prologue: make PE observe const DMA sems via tiny touch matmuls
        pro_ps = ps_x.tile([128, 512], fp32, tag="ps_x")
        nc.tensor.matmul(pro_ps[0:8, 0:8], id_t[:, 0:8], id_t[:, 0:8],
                         start=True, stop=True, skip_group_check=True)
        nc.tensor.matmul(pro_ps[0:8, 0:8], wbd_t[:, 0:8], wbd_t[:, 0:8],
                         start=True, stop=True, skip_group_check=True)
        nc.tensor.matmul(pro_ps[0:8, 0:8], bd_t[:, 0:8], bd_t[:, 0:8],
                         start=True, stop=True, skip_group_check=True)
        nc.tensor.matmul(pro_ps[0:8, 0:8], w1_t[:, 0:8], w1_t[:, 0:8],
                         start=True, stop=True, skip_group_check=True)

        # stats partials (filled per octet)
        s1_parts = [stp_p.tile([128, 1], fp32, name=f"s1p_{i}") for i in range(2 * NOCT)]
        s2_parts = [stp_p.tile([128, 1], fp32, name=f"s2p_{i}") for i in range(2 * NOCT)]

        T2_all = sb.tile([128, 2048], fp32)

        # ---- edge phase
        ed_t = None
        S_oct = None
        T1_ps = None
        for T in range(NTT):
            ch, lane = divmod(T, TCH)
            oct_, t_in_oct = divmod(T, 16)
            if lane == 0:
                ed_t = ed_p.tile([128, TCH * 1536], fp16, tag="ed")
                nc.sync.dma_start(
                    ed_t[:], ed_d[:, ch * TCH * 1536:(ch + 1) * TCH * 1536])
            base = lane * 1536
            xs_v = ed_t[:, base:base + 1024]
            ea_v = ed_t[:, base + 1024:base + 1536].rearrange(
                "p (q e) -> p q e", q=4)

            t_ps = ps_t.tile([128, 1024], fp32, tag="ps_t")
            for c2 in range(2):
                nc.tensor.matmul(t_ps[:, 512 * c2:512 * c2 + 512], id_t[:],
                                 xs_v[:, 512 * c2:512 * c2 + 512],
                                 start=True, stop=False, skip_group_check=True)
                for g in range(2):
                    q = 2 * c2 + g
                    nc.tensor.matmul(
                        t_ps[:, 512 * c2 + 256 * g:512 * c2 + 256 * (g + 1)],
                        ea_v[:, q, :], wbd_t[:],
                        start=False, stop=(g == 1), skip_group_check=True)

            # R = relu(t) (fp16, no denormals: values 0 or >= ~1e-3 typical);
            # P = e^R in [1, ~2000]; Pm = R*P.  Never exp() a large-negative
            # t: fp16 denormals trigger a ~10x DVE/engine slow path.
            R_t = r_p.tile([128, 1024], fp16, tag="rr")
            nc.vector.tensor_scalar_max(R_t[:], t_ps[:], 0.0)
            P_t = pp_p.tile([128, 1024], fp16, tag="pp")
            nc.scalar.activation(P_t[:], R_t[:], Act.Exp)
            Pm_t = pm_p.tile([128, 1024], fp16, tag="pm")
            nc.vector.tensor_tensor(out=Pm_t[:], in0=R_t[:], in1=P_t[:],
                                    op=Alu.mult)

            # ---- per-node reduction
            # S: b-folded accumulating matmuls into a per-octet accumulator
            if t_in_oct == 0:
                S_oct = ps_s.tile([128, 512], fp32, tag="ps_s",
                                  name=f"S_{oct_}")
            g8 = t_in_oct // 2
            Pv = P_t[:].rearrange("p (c b h) -> p c b h", c=2, b=8)
            for c2 in range(2):
                c = 2 * (T % 2) + c2
                for b in range(8):
                    nc.tensor.matmul(
                        S_oct[32 * c:32 * c + 32, 64 * g8:64 * (g8 + 1)],
                        bd_t[:], Pv[:, c2, b, :],
                        start=(b == 0), stop=(b == 7),
                        tile_position=(0, 32 * c), skip_group_check=True)
            # T: classic per-G [128, 512] accumulator + DVE b-reduce
            if T % 2 == 0:
                T1_ps = ps_u.tile([128, 512], fp32, tag="ps_u",
                                  name=f"T1_{T}")
            for c2 in range(2):
                c = 2 * (T % 2) + c2
                nc.tensor.matmul(T1_ps[32 * c:32 * c + 32, :], bd_t[:],
                                 Pm_t[:, 512 * c2:512 * (c2 + 1)],
                                 start=True, stop=True,
                                 tile_position=(0, 32 * c),
                                 skip_group_check=True)
            if T % 2 == 1:
                G = T // 2
                nc.vector.tensor_reduce(
                    T2_all[:, G * 64:(G + 1) * 64],
                    T1_ps[:].rearrange("p (b h) -> p h b", h=H),
                    axis=mybir.AxisListType.X, op=Alu.add)

            # ---- octet tail: combine + transpose + h1 + stats
            if t_in_oct == 15:
                o = oct_
                den = cmb_p.tile([128, 512], fp32, tag="cmb", name=f"den{o}")
                nc.vector.tensor_tensor(
                    out=den[:], in0=S_oct[:],
                    in1=corr_t[:, 512 * o:512 * (o + 1)], op=Alu.subtract)
                rec = cmb_p.tile([128, 512], fp32, tag="cmb", name=f"rec{o}")
                nc.vector.reciprocal(rec[:], den[:])
                wres = cmb_p.tile([128, 512], fp16, tag="cmb", name=f"wr{o}")
                nc.vector.tensor_tensor(
                    out=wres[:], in0=T2_all[:, 512 * o:512 * (o + 1)],
                    in1=rec[:], op=Alu.mult)
                res = cmb_p.tile([128, 512], fp16, tag="cmb", name=f"res{o}")
                nc.vector.tensor_tensor(
                    out=res[:], in0=wres[:],
                    in1=xres_t[:, 512 * o:512 * (o + 1)], op=Alu.add)

                outT = outt_p.tile([64, 1024], fp16, tag="outt")
                for j2 in range(2):
                    tr_ps = ps_x.tile([64, 512], fp16, tag="ps_x")
                    for gg in range(4):
                        g8b = 4 * j2 + gg
                        nc.tensor.transpose(
                            tr_ps[0:64, 128 * gg:128 * (gg + 1)],
                            res[:, 64 * g8b:64 * (g8b + 1)], id_t[:])
                    nc.scalar.copy(outT[:, 512 * j2:512 * (j2 + 1)],
                                   tr_ps[0:64, :])

                h1sb = h1s_p.tile([128, 1024], fp16, tag="h1s")
                for j2 in range(2):
                    h1_ps = ps_x.tile([128, 512], fp32, tag="ps_x")
                    nc.tensor.matmul(h1_ps[:], w1_t[:],
                                     outT[:, 512 * j2:512 * (j2 + 1)],
                                     start=True, stop=True,
                                     skip_group_check=True)
                    nc.scalar.activation(
                        h1sb[:, 512 * j2:512 * (j2 + 1)], h1_ps[:], Act.Copy,
                        accum_out=s1_parts[2 * o + j2][:])
                    sq = sq_p.tile([128, 512], fp16, tag="sq")
                    nc.scalar.activation(
                        sq[:], h1_ps[:], Act.Square,
                        accum_out=s2_parts[2 * o + j2][:])
                nc.scalar.dma_start(h1_d[:, 1024 * o:1024 * (o + 1)], h1sb[:])

        # ---- stats: sum the per-octet partials, write [128, 2]
        stats = sb.tile([128, 2], fp32)
        acc1 = sb.tile([128, 1], fp32)
        acc2 = sb.tile([128, 1], fp32)
        nc.vector.tensor_tensor(out=acc1[:], in0=s1_parts[0][:],
                                in1=s1_parts[1][:], op=Alu.add)
        nc.vector.tensor_tensor(out=acc2[:], in0=s2_parts[0][:],
                                in1=s2_parts[1][:], op=Alu.add)
        for i in range(2, 2 * NOCT):
            nc.vector.tensor_tensor(out=acc1[:], in0=acc1[:],
                                    in1=s1_parts[i][:], op=Alu.add)
            nc.vector.tensor_tensor(out=acc2[:], in0=acc2[:],
                                    in1=s2_parts[i][:], op=Alu.add)
        nc.vector.tensor_copy(stats[:, 0:1], acc1[:])
        nc.vector.tensor_copy(stats[:, 1:2], acc2[:])
        nc.scalar.dma_start(st_d[:], stats[:])

    _legalize_multiwaits(nc)
    return nc


# ------------------------------------------------------------ phase-2 kernel
def _build_phase2():
    fp32 = mybir.dt.float32
    fp16 = mybir.dt.float16
    Act = mybir.ActivationFunctionType
    nc = bass.Bass()

    h1_d = nc.declare_dram_parameter("h1", [128, NPC], fp16, isOutput=False)
    ss_d = nc.declare_dram_parameter("ss", [128, 2], fp32, isOutput=False)
    w2_d = nc.declare_dram_parameter("w2", [128, 64], fp16, isOutput=False)
    out_d = nc.declare_dram_parameter("out", [64, NPC], fp16, isOutput=True)

    with tile.TileContext(nc) as tc, ExitStack() as ctx:
        const = ctx.enter_context(tc.tile_pool(name="const", bufs=1))
        sb = ctx.enter_context(tc.tile_pool(name="sb", bufs=1))
        ps = ctx.enter_context(tc.tile_pool(name="ps", bufs=4, space="PSUM"))

        ss_t = const.tile([128, 2], fp32)
        nc.sync.dma_start(ss_t[:], ss_d[:])
        w2_t = const.tile([128, 64], fp16)
        nc.sync.dma_start(w2_t[:], w2_d[:])
        h1 = sb.tile([128, NPC], fp16)
        nc.sync.dma_start(h1[:], h1_d[:])

        h2 = sb.tile([128, NPC], fp16)
        nc.scalar.activation(h2[:], h1[:], Act.Relu, bias=ss_t[:, 1:2],
                             scale=ss_t[:, 0:1])
        out_sb = sb.tile([64, NPC], fp16)
        for j in range(NPC // 512):
            o_ps = ps.tile([64, 512], fp32, tag="ps")
            nc.tensor.matmul(o_ps[:], w2_t[:], h2[:, j * 512:(j + 1) * 512],
                             start=True, stop=True, skip_group_check=True)
            if j % 2 == 0:
                nc.scalar.copy(out_sb[:, j * 512:(j + 1) * 512], o_ps[:])
            else:
                nc.vector.tensor_copy(out_sb[:, j * 512:(j + 1) * 512], o_ps[:])
        nc.sync.dma_start(out_d[:], out_sb[:])

    _legalize_multiwaits(nc)
    return nc


# -------------------------------------------------------------- host staging
def _stage_core(x_c, xs_slot_c, ea_slot_c, ninv_c):
    """xs_slot_c: [npc, K, H] f32 (x[src], invalid slots = NEG_BIG)
    ea_slot_c: [npc, K, ED] f32;  ninv_c: [npc] f32.
    Returns edges [128, NTT*1536] f16, corr [128,2048] f32, xres [128,2048] f16.

    Device tile = 64 nodes: partition p = 4*(node%32) + k%4, free:
      xs block  [T, cols 0:1024)   = (c2=(node%64)//32, b=k//4, h)
      ea block  [T, cols 1024:1536) = (c2, g=b//4, (node%32, k%4)) on
                partitions (r=b%4, d)
    """
    a = xs_slot_c.reshape(NTT, 2, 32, 8, 4, H)          # [T, c2, m2, b, j, h]
    xs_part = np.ascontiguousarray(
        a.transpose(2, 4, 0, 1, 3, 5)).reshape(128, NTT, 1024)

    e = ea_slot_c.reshape(NTT, 2, 32, 2, 4, 4, ED)      # [T, c2, m2, g, r, j, d]
    ea_part = np.ascontiguousarray(
        e.transpose(4, 6, 0, 1, 3, 2, 5)).reshape(128, NTT, 512)

    edges = np.concatenate(
        [xs_part, ea_part], axis=2).reshape(128, NTT * 1536).astype(np.float16)

    # node n = 128*G + p'  (p' = n % 128)
    corr = (ninv_c.astype(np.float32) - 1e-16)[:, None] * np.ones((1, H), np.float32)
    corr_dev = np.ascontiguousarray(
        corr.reshape(NPC // 128, 128, H).transpose(1, 0, 2)).reshape(128, -1)
    xres_dev = np.ascontiguousarray(
        (x_c + 1e-7).reshape(NPC // 128, 128, H).transpose(1, 0, 2)
    ).reshape(128, -1).astype(np.float16)
    return edges, corr_dev, xres_dev


def _consts(W_edge):
    Wbd = np.zeros((128, 256), np.float16)
    for r in range(4):
        Wbd[32 * r:32 * r + 32, 64 * r:64 * r + 64] = W_edge.astype(np.float16)
    BD = np.zeros((128, 32), np.float16)
    for m in range(32):
        BD[4 * m:4 * m + 4, m] = 1.0
    ident = np.eye(128, dtype=np.float16)
    return Wbd, BD, ident


def kernel(x, edge_index, edge_attr, nbr, W_edge, W1, gamma, beta, W2):
    x = np.ascontiguousarray(np.asarray(x, dtype=np.float32))
    edge_index = np.asarray(edge_index)
    edge_attr = np.ascontiguousarray(np.asarray(edge_attr, dtype=np.float32))
    nbr = np.asarray(nbr)
    W_edge = np.ascontiguousarray(np.asarray(W_edge, dtype=np.float32))
    W1 = np.ascontiguousarray(np.asarray(W1, dtype=np.float32))
    gamma = np.asarray(gamma, dtype=np.float32)
    beta = np.asarray(beta, dtype=np.float32)
    W2 = np.ascontiguousarray(np.asarray(W2, dtype=np.float32))

    src = np.asarray(edge_index[0], dtype=np.int64)
    valid = nbr >= 0                                    # [N, K]
    expect = np.arange(E, dtype=np.int64).reshape(N, K)
    assert np.array_equal(np.where(valid, nbr, expect), expect), \
        "kernel assumes nbr[i,k] == i*K+k on valid slots"

    src_slot = src.reshape(N, K)
    xs_slot = x[src_slot]                               # host halo: [N, K, H]
    xs_slot[~valid] = NEG_BIG
    ninv = (~valid).sum(axis=1).astype(np.float32)      # [N]
    ea_slot = edge_attr.reshape(N, K, ED)

    Wbd, BD, ident = _consts(W_edge)

    if "p1" not in _compiled:
        _compiled["p1"] = _build_phase1()
        _compiled["p2"] = _build_phase2()

    in_maps = []
    for core in range(NCORES):
        sl = slice(core * NPC, (core + 1) * NPC)
        edges, corr_dev, xres_dev = _stage_core(
            x[sl], xs_slot[sl], ea_slot[sl], ninv[sl])
        in_maps.append({
            "edges": edges, "wbd": Wbd, "bd": BD, "ident": ident,
            "w1": W1.astype(np.float16), "corr": corr_dev, "xres": xres_dev,
        })

    res1 = run_bass_kernel_spmd(_compiled["p1"], in_maps,
                                core_ids=list(range(NCORES)))

    # host: combine BN stats (tiny 128-vector arithmetic), build scale/shift
    s1 = np.zeros(2 * H, np.float64)
    s2 = np.zeros(2 * H, np.float64)
    for core in range(NCORES):
        st = res1.results[core]["stats"].astype(np.float64)
        s1 += st[:, 0]
        s2 += st[:, 1]
    mean = (s1 / N).astype(np.float32)
    var = (s2 / N).astype(np.float32) - mean * mean
    scale = gamma / np.sqrt(var + 1e-5)
    shift = beta - mean * scale
    ss = np.stack([scale, shift], axis=1).astype(np.float32)  # [128, 2]

    in_maps2 = [{"h1": res1.results[core]["h1"], "ss": ss,
                 "w2": W2.astype(np.float16)}
                for core in range(NCORES)]
    res2 = run_bass_kernel_spmd(_compiled["p2"], in_maps2,
                                core_ids=list(range(NCORES)))

    out = np.empty((N, H), np.float32)
    for core in range(NCORES):
        out[core * NPC:(core + 1) * NPC] = \
            res2.results[core]["out"].astype(np.float32).T
    return out


# revision 9
# speedup vs baseline: 5.3696x; 1.0209x over previous
"""Trainium2 Bass kernel for nn_ExportableGENConv (GENConv message passing +
channelwise softmax aggregation + MLP with global-batch BatchNorm), sharded
across 8 NeuronCores.

Contract: kernel(**inputs) takes the FULL inputs of reference.setup_inputs()
and returns the FULL [32768, 64] float32 output.

Sharding: nodes (each with K=32 contiguous incoming edge slots) are split
across 8 cores. Per-edge source features x[src] are materialized host-side
during staging (the halo exchange) in fp16; the per-edge message + per-node
softmax + MLP run on device. Global BatchNorm statistics are combined on host
between two NEFF launches (in-kernel collectives fail to load under this
runtime): phase 1 produces h1 = (aggregated+x) @ W1 (fp16) plus per-core
sum/sumsq, phase 2 applies the batch-norm affine + ReLU + W2.

Math (per node i, channel h, over valid slots k):
  t = x[src] + ea @ W_edge            (invalid slots: staged x = -3e4 -> t << 0)
  reference: m = relu(t) + 1e-7; softmax over k of m; res = sum m*alpha.
  With r = relu(t):  res = (sum_k r*e^r) / (sum_k e^r + 1e-16) + 1e-7
  Device: R = relu(t) (invalid slots -> 0); P = e^R in [1, ~2e3] (invalid
  slots contribute exactly 1.0, removed via a host-staged per-node count);
  Pm = R*P (= r*e^r; invalid slots contribute exactly 0).  Computing relu
  BEFORE exp keeps every fp16 tensor denormal-free: exp() of large-negative
  t emits fp16 subnormals, which put DVE/GpSimd elementwise consumers into
  a ~15x per-element assist slow path (measured).

Device layout: tile = 64 nodes = 2048 edge slots as [128, 1024] with
partition p = (node%32)*4 + (k%4) and free = (c2 = (node%64)//32, b = k//4,
h).  Per-node k-reduction: j = k%4 via a block-diagonal ones stationary
(PE); the b dim of the P-sum via 8 accumulating 64-row matmuls into a
per-octet PSUM accumulator, the Pm-sum via a per-G PSUM tile + DVE
b-reduce (splits the load between PE and DVE).  All staged data fp16,
PSUM accumulation fp32; R alternates DVE/Act per tile for balance.
"""

import numpy as np
from contextlib import ExitStack

import concourse.bass as bass
import concourse.tile as tile
from concourse import mybir
from concourse.bass_utils import run_bass_kernel_spmd

# ---------------------------------------------------------------- constants
N, K, H, ED = 32768, 32, 64, 32
E = N * K
NCORES = 8
NPC = N // NCORES            # nodes per core = 4096
NTT = NPC // 64              # 64-node tiles per core = 64
TCH = 4                      # tiles per DMA chunk (256 nodes, 1.5 MB)
NCH = NTT // TCH             # chunks = 16
NOCT = 4                     # octets (1024 nodes each)
NEG_BIG = -30000.0           # fp16-safe "minus infinity" for invalid slots

_compiled = {}


# ------------------------------------------------------- multi-wait legalizer
def _legalize_multiwaits(nc):
    """This walrus build accepts only ONE sync wait per instruction; move the
    excess onto injected same-engine drain carriers placed immediately before
    the instruction (semantics-preserving: the engine stalls there instead)."""
    n_injected = 0
    for fn in nc.m.functions:
        for blk in fn.blocks:
            bb = blk if hasattr(blk, "instructions") else blk.bb
            insts = list(bb.instructions)
            out = []
            for inst in insts:
                si = inst.sync_info
                if si is not None and si.on_wait and len(si.on_wait) > 1:
                    waits = list(si.on_wait)
                    for w in waits[:-1]:
                        nop = mybir.InstDrain(
                            name=f"waitfix-{nc.next_id()}", ins=[], outs=[]
                        )
                        nop.engine = inst.engine
                        nop.sync_info = mybir.SyncInfo(on_wait=[w], on_update=[])
                        nc.register_instruction(nop, overwrite=True)
                        out.append(nop)
                        n_injected += 1
                    inst.sync_info = mybir.SyncInfo(
                        on_wait=[waits[-1]], on_update=list(si.on_update or [])
                    )
                out.append(inst)
            if len(out) != len(insts):
                bb.instructions = out
    return n_injected


# ------------------------------------------------------------ phase-1 kernel
def _build_phase1():
    fp32 = mybir.dt.float32
    fp16 = mybir.dt.float16
    Act = mybir.ActivationFunctionType
    Alu = mybir.AluOpType
    nc = bass.Bass()

    ed_d = nc.declare_dram_parameter("edges", [128, NTT * 1536], fp16, isOutput=False)
    wbd_d = nc.declare_dram_parameter("wbd", [128, 256], fp16, isOutput=False)
    bd_d = nc.declare_dram_parameter("bd", [128, 32], fp16, isOutput=False)
    id_d = nc.declare_dram_parameter("ident", [128, 128], fp16, isOutput=False)
    w1_d = nc.declare_dram_parameter("w1", [64, 128], fp16, isOutput=False)
    corr_d = nc.declare_dram_parameter("corr", [128, 2048], fp32, isOutput=False)
    xres_d = nc.declare_dram_parameter("xres", [128, 2048], fp16, isOutput=False)
    h1_d = nc.declare_dram_parameter("h1", [128, NPC], fp16, isOutput=True)
    st_d = nc.declare_dram_parameter("stats", [128, 2], fp32, isOutput=True)

    with tile.TileContext(nc) as tc, ExitStack() as ctx:
        const = ctx.enter_context(tc.tile_pool(name="const", bufs=1))
        sb = ctx.enter_context(tc.tile_pool(name="sb", bufs=1))
        ed_p = ctx.enter_context(tc.tile_pool(name="ed", bufs=3))
        r_p = ctx.enter_context(tc.tile_pool(name="rr", bufs=3))
        pp_p = ctx.enter_context(tc.tile_pool(name="pp", bufs=3))
        pm_p = ctx.enter_context(tc.tile_pool(name="pm", bufs=3))
        cmb_p = ctx.enter_context(tc.tile_pool(name="cmb", bufs=2))
        outt_p = ctx.enter_context(tc.tile_pool(name="outt", bufs=2))
        h1s_p = ctx.enter_context(tc.tile_pool(name="h1s", bufs=2))
        sq_p = ctx.enter_context(tc.tile_pool(name="sq", bufs=2))
        stp_p = ctx.enter_context(tc.tile_pool(name="stp", bufs=1))
        ps_t = ctx.enter_context(tc.tile_pool(name="ps_t", bufs=2, space="PSUM"))
        ps_s = ctx.enter_context(tc.tile_pool(name="ps_s", bufs=1, space="PSUM"))
        ps_u = ctx.enter_context(tc.tile_pool(name="ps_u", bufs=1, space="PSUM"))
        ps_x = ctx.enter_context(tc.tile_pool(name="ps_x", bufs=2, space="PSUM"))

        wbd_t = const.tile([128, 256], fp16)
        nc.sync.dma_start(wbd_t[:], wbd_d[:])
        bd_t = const.tile([128, 32], fp16)
        nc.sync.dma_start(bd_t[:], bd_d[:])
        id_t = const.tile([128, 128], fp16)
        nc.sync.dma_start(id_t[:], id_d[:])
        w1_t = const.tile([64, 128], fp16)
        nc.sync.dma_start(w1_t[:], w1_d[:])
        corr_t = const.tile([128, 2048], fp32)
        nc.sync.dma_start(corr_t[:], corr_d[:])
        xres_t = const.tile([128, 2048], fp16)
        nc.sync.dma_start(xres_t[:], xres_d[:])

        # prologue: make PE observe const DMA sems via tiny touch matmuls
        pro_ps = ps_x.tile([128, 512], fp32, tag="ps_x")
        nc.tensor.matmul(pro_ps[0:8, 0:8], id_t[:, 0:8], id_t[:, 0:8],
                         start=True, stop=True, skip_group_check=True)
        nc.tensor.matmul(pro_ps[0:8, 0:8], wbd_t[:, 0:8], wbd_t[:, 0:8],
                         start=True, stop=True, skip_group_check=True)
        nc.tensor.matmul(pro_ps[0:8, 0:8], bd_t[:, 0:8], bd_t[:, 0:8],
                         start=True, stop=True, skip_group_check=True)
        nc.tensor.matmul(pro_ps[0:8, 0:8], w1_t[:, 0:8], w1_t[:, 0:8],
                         start=True, stop=True, skip_group_check=True)

        # stats partials (filled per octet)
        s1_parts = [stp_p.tile([128, 1], fp32, name=f"s1p_{i}") for i in range(2 * NOCT)]
        s2_parts = [stp_p.tile([128, 1], fp32, name=f"s2p_{i}") for i in range(2 * NOCT)]

        T2_all = sb.tile([128, 2048], fp32)

        # ---- edge phase
        ed_t = None
        S_oct = None
        T1_ps = None
        for T in range(NTT):
            ch, lane = divmod(T, TCH)
            oct_, t_in_oct = divmod(T, 16)
            if lane == 0:
                ed_t = ed_p.tile([128, TCH * 1536], fp16, tag="ed")
                if ch == 0:
                    # split the first chunk's load so tile-0 compute can
                    # start after a quarter of the bytes land
                    nc.sync.dma_start(ed_t[:, 0:1536], ed_d[:, 0:1536])
                    nc.sync.dma_start(ed_t[:, 1536:TCH * 1536],
                                      ed_d[:, 1536:TCH * 1536])
                else:
                    nc.sync.dma_start(
                        ed_t[:], ed_d[:, ch * TCH * 1536:(ch + 1) * TCH * 1536])
            base = lane * 1536
            xs_v = ed_t[:, base:base + 1024]
            ea_v = ed_t[:, base + 1024:base + 1536].rearrange(
                "p (q e) -> p q e", q=4)

            t_ps = ps_t.tile([128, 1024], fp32, tag="ps_t")
            for c2 in range(2):
                nc.tensor.matmul(t_ps[:, 512 * c2:512 * c2 + 512], id_t[:],
                                 xs_v[:, 512 * c2:512 * c2 + 512],
                                 start=True, stop=False, skip_group_check=True)
                for g in range(2):
                    q = 2 * c2 + g
                    nc.tensor.matmul(
                        t_ps[:, 512 * c2 + 256 * g:512 * c2 + 256 * (g + 1)],
                        ea_v[:, q, :], wbd_t[:],
                        start=False, stop=(g == 1), skip_group_check=True)

            # R = relu(t) (fp16, no denormals: values 0 or >= ~1e-3 typical);
            # P = e^R in [1, ~2000]; Pm = R*P.  Never exp() a large-negative
            # t: fp16 denormals trigger a ~10x DVE/engine slow path.
            R_t = r_p.tile([128, 1024], fp16, tag="rr")
            if T % 2 == 0:
                nc.vector.tensor_scalar_max(R_t[:], t_ps[:], 0.0)
            else:
                nc.scalar.activation(R_t[:], t_ps[:], Act.Relu)
            P_t = pp_p.tile([128, 1024], fp16, tag="pp")
            nc.scalar.activation(P_t[:], R_t[:], Act.Exp)
            Pm_t = pm_p.tile([128, 1024], fp16, tag="pm")
            nc.vector.tensor_tensor(out=Pm_t[:], in0=R_t[:], in1=P_t[:],
                                    op=Alu.mult)

            # ---- per-node reduction
            # S: b-folded accumulating matmuls into a per-octet accumulator
            if t_in_oct == 0:
                S_oct = ps_s.tile([128, 512], fp32, tag="ps_s",
                                  name=f"S_{oct_}")
            g8 = t_in_oct // 2
            Pv = P_t[:].rearrange("p (c b h) -> p c b h", c=2, b=8)
            for c2 in range(2):
                c = 2 * (T % 2) + c2
                for b in range(8):
                    nc.tensor.matmul(
                        S_oct[32 * c:32 * c + 32, 64 * g8:64 * (g8 + 1)],
                        bd_t[:], Pv[:, c2, b, :],
                        start=(b == 0), stop=(b == 7),
                        tile_position=(0, 32 * c), skip_group_check=True)
            # T: classic per-G [128, 512] accumulator + DVE b-reduce
            if T % 2 == 0:
                T1_ps = ps_u.tile([128, 512], fp32, tag="ps_u",
                                  name=f"T1_{T}")
            for c2 in range(2):
                c = 2 * (T % 2) + c2
                nc.tensor.matmul(T1_ps[32 * c:32 * c + 32, :], bd_t[:],
                                 Pm_t[:, 512 * c2:512 * (c2 + 1)],
                                 start=True, stop=True,
                                 tile_position=(0, 32 * c),
                                 skip_group_check=True)
            if T % 2 == 1:
                G = T // 2
                nc.vector.tensor_reduce(
                    T2_all[:, G * 64:(G + 1) * 64],
                    T1_ps[:].rearrange("p (b h) -> p h b", h=H),
                    axis=mybir.AxisListType.X, op=Alu.add)

            # ---- octet tail: combine + transpose + h1 + stats
            if t_in_oct == 15:
                o = oct_
                den = cmb_p.tile([128, 512], fp32, tag="cmb", name=f"den{o}")
                nc.vector.tensor_tensor(
                    out=den[:], in0=S_oct[:],
                    in1=corr_t[:, 512 * o:512 * (o + 1)], op=Alu.subtract)
                rec = cmb_p.tile([128, 512], fp32, tag="cmb", name=f"rec{o}")
                nc.vector.reciprocal(rec[:], den[:])
                wres = cmb_p.tile([128, 512], fp16, tag="cmb", name=f"wr{o}")
                nc.vector.tensor_tensor(
                    out=wres[:], in0=T2_all[:, 512 * o:512 * (o + 1)],
                    in1=rec[:], op=Alu.mult)
                res = cmb_p.tile([128, 512], fp16, tag="cmb", name=f"res{o}")
                nc.vector.tensor_tensor(
                    out=res[:], in0=wres[:],
                    in1=xres_t[:, 512 * o:512 * (o + 1)], op=Alu.add)

                outT = outt_p.tile([64, 1024], fp16, tag="outt")
                for j2 in range(2):
                    tr_ps = ps_x.tile([64, 512], fp16, tag="ps_x")
                    for gg in range(4):
                        g8b = 4 * j2 + gg
                        nc.tensor.transpose(
                            tr_ps[0:64, 128 * gg:128 * (gg + 1)],
                            res[:, 64 * g8b:64 * (g8b + 1)], id_t[:])
                    nc.scalar.copy(outT[:, 512 * j2:512 * (j2 + 1)],
                                   tr_ps[0:64, :])

                h1sb = h1s_p.tile([128, 1024], fp16, tag="h1s")
                for j2 in range(2):
                    h1_ps = ps_x.tile([128, 512], fp32, tag="ps_x")
                    nc.tensor.matmul(h1_ps[:], w1_t[:],
                                     outT[:, 512 * j2:512 * (j2 + 1)],
                                     start=True, stop=True,
                                     skip_group_check=True)
                    nc.scalar.activation(
                        h1sb[:, 512 * j2:512 * (j2 + 1)], h1_ps[:], Act.Copy,
                        accum_out=s1_parts[2 * o + j2][:])
                    sq = sq_p.tile([128, 512], fp16, tag="sq")
                    nc.scalar.activation(
                        sq[:], h1_ps[:], Act.Square,
                        accum_out=s2_parts[2 * o + j2][:])
                nc.scalar.dma_start(h1_d[:, 1024 * o:1024 * (o + 1)], h1sb[:])

        # ---- stats: sum the per-octet partials, write [128, 2]
        stats = sb.tile([128, 2], fp32)
        acc1 = sb.tile([128, 1], fp32)
        acc2 = sb.tile([128, 1], fp32)
        nc.vector.tensor_tensor(out=acc1[:], in0=s1_parts[0][:],
                                in1=s1_parts[1][:], op=Alu.add)
        nc.vector.tensor_tensor(out=acc2[:], in0=s2_parts[0][:],
                                in1=s2_parts[1][:], op=Alu.add)
        for i in range(2, 2 * NOCT):
            nc.vector.tensor_tensor(out=acc1[:], in0=acc1[:],
                                    in1=s1_parts[i][:], op=Alu.add)
            nc.vector.tensor_tensor(out=acc2[:], in0=acc2[:],
                                    in1=s2_parts[i][:], op=Alu.add)
        nc.vector.tensor_copy(stats[:, 0:1], acc1[:])
        nc.vector.tensor_copy(stats[:, 1:2], acc2[:])
        nc.scalar.dma_start(st_d[:], stats[:])

    _legalize_multiwaits(nc)
    return nc


# ------------------------------------------------------------ phase-2 kernel
def _build_phase2():
    fp32 = mybir.dt.float32
    fp16 = mybir.dt.float16
    Act = mybir.ActivationFunctionType
    nc = bass.Bass()

    h1_d = nc.declare_dram_parameter("h1", [128, NPC], fp16, isOutput=False)
    ss_d = nc.declare_dram_parameter("ss", [128, 2], fp32, isOutput=False)
    w2_d = nc.declare_dram_parameter("w2", [128, 64], fp16, isOutput=False)
    out_d = nc.declare_dram_parameter("out", [64, NPC], fp16, isOutput=True)

    with tile.TileContext(nc) as tc, ExitStack() as ctx:
        const = ctx.enter_context(tc.tile_pool(name="const", bufs=1))
        sb = ctx.enter_context(tc.tile_pool(name="sb", bufs=1))
        ps = ctx.enter_context(tc.tile_pool(name="ps", bufs=4, space="PSUM"))

        ss_t = const.tile([128, 2], fp32)
        nc.sync.dma_start(ss_t[:], ss_d[:])
        w2_t = const.tile([128, 64], fp16)
        nc.sync.dma_start(w2_t[:], w2_d[:])
        h1 = sb.tile([128, NPC], fp16)
        nc.sync.dma_start(h1[:], h1_d[:])

        h2 = sb.tile([128, NPC], fp16)
        nc.scalar.activation(h2[:], h1[:], Act.Relu, bias=ss_t[:, 1:2],
                             scale=ss_t[:, 0:1])
        out_sb = sb.tile([64, NPC], fp16)
        for j in range(NPC // 512):
            o_ps = ps.tile([64, 512], fp32, tag="ps")
            nc.tensor.matmul(o_ps[:], w2_t[:], h2[:, j * 512:(j + 1) * 512],
                             start=True, stop=True, skip_group_check=True)
            if j % 2 == 0:
                nc.scalar.copy(out_sb[:, j * 512:(j + 1) * 512], o_ps[:])
            else:
                nc.vector.tensor_copy(out_sb[:, j * 512:(j + 1) * 512], o_ps[:])
        nc.sync.dma_start(out_d[:], out_sb[:])

    _legalize_multiwaits(nc)
    return nc


# -------------------------------------------------------------- host staging
def _stage_core(x_c, xs_slot_c, ea_slot_c, ninv_c):
    """xs_slot_c: [npc, K, H] f32 (x[src], invalid slots = NEG_BIG)
    ea_slot_c: [npc, K, ED] f32;  ninv_c: [npc] f32.
    Returns edges [128, NTT*1536] f16, corr [128,2048] f32, xres [128,2048] f16.

    Device tile = 64 nodes: partition p = 4*(node%32) + k%4, free:
      xs block  [T, cols 0:1024)   = (c2=(node%64)//32, b=k//4, h)
      ea block  [T, cols 1024:1536) = (c2, g=b//4, (node%32, k%4)) on
                partitions (r=b%4, d)
    """
    a = xs_slot_c.reshape(NTT, 2, 32, 8, 4, H)          # [T, c2, m2, b, j, h]
    xs_part = np.ascontiguousarray(
        a.transpose(2, 4, 0, 1, 3, 5)).reshape(128, NTT, 1024)

    e = ea_slot_c.reshape(NTT, 2, 32, 2, 4, 4, ED)      # [T, c2, m2, g, r, j, d]
    ea_part = np.ascontiguousarray(
        e.transpose(4, 6, 0, 1, 3, 2, 5)).reshape(128, NTT, 512)

    edges = np.concatenate(
        [xs_part, ea_part], axis=2).reshape(128, NTT * 1536).astype(np.float16)

    # node n = 128*G + p'  (p' = n % 128)
    corr = (ninv_c.astype(np.float32) - 1e-16)[:, None] * np.ones((1, H), np.float32)
    corr_dev = np.ascontiguousarray(
        corr.reshape(NPC // 128, 128, H).transpose(1, 0, 2)).reshape(128, -1)
    xres_dev = np.ascontiguousarray(
        (x_c + 1e-7).reshape(NPC // 128, 128, H).transpose(1, 0, 2)
    ).reshape(128, -1).astype(np.float16)
    return edges, corr_dev, xres_dev


def _consts(W_edge):
    Wbd = np.zeros((128, 256), np.float16)
    for r in range(4):
        Wbd[32 * r:32 * r + 32, 64 * r:64 * r + 64] = W_edge.astype(np.float16)
    BD = np.zeros((128, 32), np.float16)
    for m in range(32):
        BD[4 * m:4 * m + 4, m] = 1.0
    ident = np.eye(128, dtype=np.float16)
    return Wbd, BD, ident


def kernel(x, edge_index, edge_attr, nbr, W_edge, W1, gamma, beta, W2):
    x = np.ascontiguousarray(np.asarray(x, dtype=np.float32))
    edge_index = np.asarray(edge_index)
    edge_attr = np.ascontiguousarray(np.asarray(edge_attr, dtype=np.float32))
    nbr = np.asarray(nbr)
    W_edge = np.ascontiguousarray(np.asarray(W_edge, dtype=np.float32))
    W1 = np.ascontiguousarray(np.asarray(W1, dtype=np.float32))
    gamma = np.asarray(gamma, dtype=np.float32)
    beta = np.asarray(beta, dtype=np.float32)
    W2 = np.ascontiguousarray(np.asarray(W2, dtype=np.float32))

    src = np.asarray(edge_index[0], dtype=np.int64)
    valid = nbr >= 0                                    # [N, K]
    expect = np.arange(E, dtype=np.int64).reshape(N, K)
    assert np.array_equal(np.where(valid, nbr, expect), expect), \
        "kernel assumes nbr[i,k] == i*K+k on valid slots"

    src_slot = src.reshape(N, K)
    xs_slot = x[src_slot]                               # host halo: [N, K, H]
    xs_slot[~valid] = NEG_BIG
    ninv = (~valid).sum(axis=1).astype(np.float32)      # [N]
    ea_slot = edge_attr.reshape(N, K, ED)

    Wbd, BD, ident = _consts(W_edge)

    if "p1" not in _compiled:
        _compiled["p1"] = _build_phase1()
        _compiled["p2"] = _build_phase2()

    in_maps = []
    for core in range(NCORES):
        sl = slice(core * NPC, (core + 1) * NPC)
        edges, corr_dev, xres_dev = _stage_core(
            x[sl], xs_slot[sl], ea_slot[sl], ninv[sl])
        in_maps.append({
            "edges": edges, "wbd": Wbd, "bd": BD, "ident": ident,
            "w1": W1.astype(np.float16), "corr": corr_dev, "xres": xres_dev,
        })

    res1 = run_bass_kernel_spmd(_compiled["p1"], in_maps,
                                core_ids=list(range(NCORES)))

    # host: combine BN stats (tiny 128-vector arithmetic), build scale/shift
    s1 = np.zeros(2 * H, np.float64)
    s2 = np.zeros(2 * H, np.float64)
    for core in range(NCORES):
        st = res1.results[core]["stats"].astype(np.float64)
        s1 += st[:, 0]
        s2 += st[:, 1]
    mean = (s1 / N).astype(np.float32)
    var = (s2 / N).astype(np.float32) - mean * mean
    scale = gamma / np.sqrt(var + 1e-5)
    shift = beta - mean * scale
    ss = np.stack([scale, shift], axis=1).astype(np.float32)  # [128, 2]

    in_maps2 = [{"h1": res1.results[core]["h1"], "ss": ss,
                 "w2": W2.astype(np.float16)}
                for core in range(NCORES)]
    res2 = run_bass_kernel_spmd(_compiled["p2"], in_maps2,
                                core_ids=list(range(NCORES)))

    out = np.empty((N, H), np.float32)
    for core in range(NCORES):
        out[core * NPC:(core + 1) * NPC] = \
            res2.results[core]["out"].astype(np.float32).T
    return out
